# revision 13
# baseline (speedup 1.0000x reference)
import numpy as np
import concourse.bacc as bacc
import concourse.mybir as mybir
from concourse.tile import TileContext
from concourse.bass_utils import run_bass_kernel_spmd

F32 = mybir.dt.float32
F32R = mybir.dt.float32r
AF = mybir.ActivationFunctionType

EPS = 2.220446049250313e-16
BN_EPS = 1e-3
C_WAY, N_SUP, N_QRY = 5, 5, 30
K_TOP = 20
NIMG = 175
NCORES = 8
PCORE = 22  # images per core (last core: 21 real + 1 zero dummy)

_NC_CACHE = {}


def _r11(x):
    u = np.ascontiguousarray(x, np.float32).view(np.uint32)
    r = ((u + np.uint32(1 << 11)) >> 12) << 12
    return r.view(np.float32)


def _build_bass():
    nc = bacc.Bacc("TRN2", target_bir_lowering=False, debug=False, num_devices=1)
    X27 = nc.dram_tensor("x27", [27, PCORE * 7396], F32R, kind="ExternalInput")
    W1 = nc.dram_tensor("w1", [27, 64], F32R, kind="ExternalInput")
    W2 = nc.dram_tensor("w2", [64, 9 * 64], F32R, kind="ExternalInput")
    W3 = nc.dram_tensor("w3", [64, 9 * 64], F32R, kind="ExternalInput")
    W4 = nc.dram_tensor("w4", [64, 9 * 64], F32R, kind="ExternalInput")
    BIAS = nc.dram_tensor("bias", [64, 4], F32, kind="ExternalOutput" if False else "ExternalInput")
    EMB = nc.dram_tensor("emb", [64, PCORE * 25], F32, kind="ExternalOutput")

    L1 = 7396            # 86*86 padded image
    L2 = 44 * 44         # conv2 input padded (42+2)^2
    L3 = 23 * 23         # conv3 input padded (21+2)^2
    L4 = 12 * 12         # conv4 input padded (10+2)^2
    NB4 = 24             # conv4 batches of 4

    with TileContext(nc) as tc:
        with tc.tile_pool(name="const", bufs=1) as cp, \
             tc.tile_pool(name="act", bufs=1) as ap, \
             tc.tile_pool(name="work", bufs=3) as wp, \
             tc.tile_pool(name="ps", bufs=2, space="PSUM") as pp:
            w1 = cp.tile([27, 64], F32R)
            nc.sync.dma_start(w1[:], W1.ap())
            w2 = cp.tile([64, 9 * 64], F32R)
            nc.sync.dma_start(w2[:], W2.ap())
            w3 = cp.tile([64, 9 * 64], F32R)
            nc.sync.dma_start(w3[:], W3.ap())
            w4 = cp.tile([64, 9 * 64], F32R)
            nc.sync.dma_start(w4[:], W4.ap())
            bias = cp.tile([64, 4], F32)
            nc.sync.dma_start(bias[:], BIAS.ap())
            zro = cp.tile([64, 1], F32)
            nc.vector.memset(zro[:], 0.0)

            x3 = ap.tile([64, PCORE * L3 + 600], F32R)
            nc.scalar.activation(x3[:], zro[:].to_broadcast([64, PCORE * L3 + 600]), AF.Relu)
            x4 = ap.tile([64, NB4 * L4 + 256], F32R)
            nc.scalar.activation(x4[:], zro[:].to_broadcast([64, NB4 * L4 + 256]), AF.Relu)
            embt = ap.tile([64, PCORE * 25], F32)

            # ---- block 1 + block 2 per image ----
            for g in range(PCORE):
                x2 = wp.tile([64, L2 + 64], F32R, tag="x2")
                nc.scalar.activation(x2[:], zro[:].to_broadcast([64, L2 + 64]), AF.Relu)
                st = wp.tile([27, L1], F32R, tag="stack")
                nc.sync.dma_start(st[:], X27.ap()[:, g * L1:(g + 1) * L1])
                for t in range(14):  # 14 tiles of 6 conv1-out rows
                    ps1 = pp.tile([64, 504], F32, tag="ps")
                    off = 87 + (6 * t) * 86
                    nc.tensor.matmul(
                        ps1[:],
                        w1[:], st[:, off:off + 516].rearrange("p (r c) -> p r c", r=6, c=86)[:, :, 0:84],
                        start=True, stop=True)
                    tmp = wp.tile([64, 126], F32, tag="tmp1")
                    nc.vector.tensor_reduce(
                        tmp[:],
                        ps1[:].rearrange("p (gy wy gx wx) -> p gy gx wy wx",
                                         gy=3, wy=2, gx=42, wx=2),
                        axis=mybir.AxisListType.XY, op=mybir.AluOpType.max)
                    dst = x2[:, (1 + 3 * t) * 44 + 1:]
                    nc.scalar.activation(
                        dst[:, 0:132].rearrange("p (r c) -> p r c", r=3, c=44)[:, :, 0:42],
                        tmp[:].rearrange("p (r c) -> p r c", r=3, c=42),
                        AF.Relu, bias=bias[:, 0:1])
                # conv2: tiles of (10,10,10,12) out rows, single MM per tap
                r0 = 0
                for rows in (10, 10, 10, 12):
                    ps2 = pp.tile([64, 504], F32, tag="ps")
                    for tap in range(9):
                        dy, dx = tap // 3, tap % 3
                        off = (r0 + dy) * 44 + dx
                        nc.tensor.matmul(
                            ps2[:, 0:rows * 42],
                            w2[:, tap * 64:(tap + 1) * 64],
                            x2[:, off:off + rows * 44].rearrange("p (r c) -> p r c", r=rows, c=44)[:, :, 0:42],
                            start=(tap == 0), stop=(tap == 8))
                    tmp2 = wp.tile([64, 126], F32, tag="tmp2")
                    nc.vector.tensor_reduce(
                        tmp2[:, 0:(rows // 2) * 21],
                        ps2[:, 0:rows * 42].rearrange("p (gy wy gx wx) -> p gy gx wy wx",
                                                      gy=rows // 2, wy=2, gx=21, wx=2),
                        axis=mybir.AxisListType.XY, op=mybir.AluOpType.max)
                    dst = x3[:, g * L3 + (1 + r0 // 2) * 23 + 1:]
                    nc.scalar.activation(
                        dst[:, 0:(rows // 2) * 23].rearrange("p (r c) -> p r c", r=rows // 2, c=23)[:, :, 0:21],
                        tmp2[:, 0:(rows // 2) * 21].rearrange("p (r c) -> p r c", r=rows // 2, c=21),
                        AF.Relu, bias=bias[:, 1:2])
                    r0 += rows

            # ---- block 3: image pairs, contiguous psum packing ----
            for gp in range(11):
                psA = pp.tile([64, 484], F32, tag="ps3a")  # out rows 0-10, 22 cols, 2 imgs
                psB = pp.tile([64, 484], F32, tag="ps3b")  # out rows 10-20
                for tap in range(9):
                    dy, dx = tap // 3, tap % 3
                    off0 = (2 * gp) * L3 + dy * 23 + dx
                    nc.tensor.matmul(
                        psA[:],
                        w3[:, tap * 64:(tap + 1) * 64],
                        x3[:, off0:off0 + 2 * L3].rearrange("p (i x) -> p i x", i=2, x=L3)
                            .rearrange("p i (r c) -> p i r c", r=23, c=23)[:, :, 0:11, 0:22],
                        start=(tap == 0), stop=(tap == 8))
                    nc.tensor.matmul(
                        psB[:],
                        w3[:, tap * 64:(tap + 1) * 64],
                        x3[:, off0 + 10 * 23:off0 + 10 * 23 + 2 * L3]
                            .rearrange("p (i x) -> p i x", i=2, x=L3)
                            .rearrange("p i (r c) -> p i r c", r=23, c=23)[:, :, 0:11, 0:22],
                        start=(tap == 0), stop=(tap == 8))
                for i in range(2):
                    g = 2 * gp + i
                    tmp3 = wp.tile([64, 50], F32, tag="tmp3")
                    nc.vector.tensor_reduce(
                        tmp3[:],
                        psA[:, i * 242:i * 242 + 220]
                            .rearrange("p (r c) -> p r c", r=10, c=22)[:, :, 0:20]
                            .rearrange("p (gy wy) (gx wx) -> p gy gx wy wx", wy=2, wx=2),
                        axis=mybir.AxisListType.XY, op=mybir.AluOpType.max)
                    dst = x4[:, g * L4 + 1 * 12 + 1:]
                    nc.scalar.activation(
                        dst[:, 0:60].rearrange("p (r c) -> p r c", r=5, c=12)[:, :, 0:10],
                        tmp3[:].rearrange("p (r c) -> p r c", r=5, c=10),
                        AF.Relu, bias=bias[:, 2:3])
                    tmp3b = wp.tile([64, 50], F32, tag="tmp3b")
                    nc.vector.tensor_reduce(
                        tmp3b[:],
                        psB[:, i * 242:i * 242 + 220]
                            .rearrange("p (r c) -> p r c", r=10, c=22)[:, :, 0:20]
                            .rearrange("p (gy wy) (gx wx) -> p gy gx wy wx", wy=2, wx=2),
                        axis=mybir.AxisListType.XY, op=mybir.AluOpType.max)
                    dst = x4[:, g * L4 + 6 * 12 + 1:]
                    nc.scalar.activation(
                        dst[:, 0:60].rearrange("p (r c) -> p r c", r=5, c=12)[:, :, 0:10],
                        tmp3b[:].rearrange("p (r c) -> p r c", r=5, c=10),
                        AF.Relu, bias=bias[:, 2:3])

            # ---- block 4: groups of 4 images, contiguous psum ----
            for gq in range(6):
                ps4full = pp.tile([64, 504], F32, tag="ps")
                ps4 = ps4full[:, 0:400]
                for tap in range(9):
                    dy, dx = tap // 3, tap % 3
                    off0 = (4 * gq) * L4 + dy * 12 + dx
                    nc.tensor.matmul(
                        ps4,
                        w4[:, tap * 64:(tap + 1) * 64],
                        x4[:, off0:off0 + 4 * L4].rearrange("p (i x) -> p i x", i=4, x=L4)
                                    .rearrange("p i (r c) -> p i r c", r=12, c=12)[:, :, 0:10, 0:10],
                        start=(tap == 0), stop=(tap == 8))
                for i in range(4):
                    g = 4 * gq + i
                    if g >= PCORE:
                        continue
                    tmp4 = wp.tile([64, 25], F32, tag="tmp4")
                    nc.vector.tensor_reduce(
                        tmp4[:],
                        ps4full[:, i * 100:(i + 1) * 100].rearrange("p (gy wy gx wx) -> p gy gx wy wx",
                                                   gy=5, wy=2, gx=5, wx=2),
                        axis=mybir.AxisListType.XY, op=mybir.AluOpType.max)
                    nc.scalar.activation(embt[:, g * 25:(g + 1) * 25], tmp4[:],
                                         AF.Relu, bias=bias[:, 3:4])
            nc.sync.dma_start(EMB.ap(), embt[:])
    nc.finalize()
    return nc


def _prep_core_inputs(imgs_cm, wl, biases):
    # imgs_cm: [PCORE, 3, 86, 86] padded channel-major float32
    shifts = [dy * 86 + dx for dy in (-1, 0, 1) for dx in (-1, 0, 1)]
    flat = imgs_cm.reshape(PCORE, 3, 7396)
    x27 = np.zeros((27, PCORE * 7396), np.float32)
    for t, d in enumerate(shifts):
        for c in range(3):
            row = np.zeros((PCORE, 7396), np.float32)
            if d >= 0:
                row[:, 0:7396 - d] = flat[:, c, d:]
            else:
                row[:, -d:] = flat[:, c, 0:7396 + d]
            x27[3 * t + c] = row.reshape(-1)
    return {"x27": _r11(x27), "w1": wl[0], "w2": wl[1], "w3": wl[2], "w4": wl[3],
            "bias": biases}


def _fold_bn(p):
    w, b, g, be, mm, mv = [np.asarray(x, np.float64) for x in p]
    s = g / np.sqrt(mv + BN_EPS)
    wf = (w * s).astype(np.float32)
    bf = ((b - mm) * s + be).astype(np.float32)
    return wf, bf


def _graph_host(emb, params):
    # emb: [175, 1600] float64; returns (proto, ce, acc) float32
    enc, rel, alpha = params["enc"], params["rel"], float(np.asarray(params["alpha"]))
    emb_s, emb_q = emb[:25], emb[25:175]

    def pair_sqmean(a, b):
        d = a.shape[1]
        sa = np.sum(a * a, axis=1)
        sb = np.sum(b * b, axis=1)
        return np.maximum(sa[:, None] + sb[None, :] - 2.0 * (a @ b.T), 0.0) / d

    z = emb_s.reshape(C_WAY, N_SUP, -1).mean(axis=1)
    lo = -pair_sqmean(emb_q, z)
    lo = lo - lo.max(axis=1, keepdims=True)
    lp = lo - np.log(np.exp(lo).sum(axis=1, keepdims=True))
    lp = lp.reshape(C_WAY, N_QRY, C_WAY)
    cls = np.arange(C_WAY)
    proto = -np.mean(lp[cls[:, None], np.arange(N_QRY)[None, :], cls[:, None]])

    def conv3x3(x, w):
        N, H, W, Ci = x.shape
        xp = np.zeros((N, H + 2, W + 2, Ci), x.dtype)
        xp[:, 1:H + 1, 1:W + 1, :] = x
        out = np.zeros((N, H, W, w.shape[3]), x.dtype)
        for dy in range(3):
            for dx in range(3):
                out += xp[:, dy:dy + H, dx:dx + W, :] @ w[dy, dx]
        return out

    def block(x, p, pool_pad):
        w, b, g_, be, mm, mv = [np.asarray(t, np.float64) for t in p]
        x = conv3x3(x, w) + b
        x = g_ * (x - mm) / np.sqrt(mv + BN_EPS) + be
        x = np.maximum(x, 0)
        N, H, W, C = x.shape
        if pool_pad == "SAME" and (H % 2 or W % 2):
            xp = np.full((N, H + H % 2, W + W % 2, C), -np.inf, x.dtype)
            xp[:, :H, :W, :] = x
            x = xp
            N, H, W, C = x.shape
        x = x[:, :H // 2 * 2, :W // 2 * 2, :].reshape(N, H // 2, 2, W // 2, 2, C)
        return x.max(axis=(2, 4))

    xr = emb.reshape(-1, 5, 5, 64)
    xr = block(xr, rel["c1"], "SAME")
    xr = block(xr, rel["c2"], "SAME")
    xr = xr.reshape(xr.shape[0], -1)
    xr = np.maximum(xr @ np.asarray(rel["w1"], np.float64) + np.asarray(rel["b1"], np.float64), 0)
    sigma = 1 / (1 + np.exp(-(xr @ np.asarray(rel["w2"], np.float64) + np.asarray(rel["b2"], np.float64))))

    xn = emb / (sigma + EPS)
    Wm = np.exp(-pair_sqmean(xn, xn) / 2.0)
    idx = np.argsort(-Wm, axis=1, kind="stable")[:, :K_TOP]
    mask = np.zeros((NIMG, NIMG))
    mask[np.arange(NIMG)[:, None], idx] = 1.0
    mask = ((mask + mask.T) > 0).astype(np.float64)
    Wm = mask * Wm
    Dg = Wm.sum(axis=0)
    dsi = np.sqrt(1.0 / (Dg + EPS))
    S = dsi[:, None] * Wm * dsi[None, :]
    ys = np.repeat(np.eye(C_WAY), N_SUP, axis=0)
    y = np.concatenate([ys, np.full((150, C_WAY), EPS)], axis=0)
    F = np.linalg.solve(np.eye(NIMG) - alpha * S + EPS, y)
    label = np.argmax(F, axis=1)
    Fs = np.exp(F - F.max(axis=1, keepdims=True))
    Fs /= Fs.sum(axis=1, keepdims=True)
    gt = np.repeat(np.arange(C_WAY), N_QRY)
    y_oh = np.concatenate([ys, np.eye(C_WAY)[gt]], axis=0)
    ce = -np.mean(np.sum(y_oh * np.log(Fs + EPS), axis=1))
    acc = np.mean(label[25:] == gt)
    return (np.float32(proto), np.float32(ce), np.float32(acc))


def kernel(s, q, params):
    s = np.asarray(s, np.float32)
    q = np.asarray(q, np.float32)
    enc = params["enc"]
    imgs = np.concatenate([s.reshape(-1, 84, 84, 3), q.reshape(-1, 84, 84, 3)], axis=0)
    pad = np.zeros((NCORES * PCORE, 3, 86, 86), np.float32)
    pad[:NIMG, :, 1:85, 1:85] = imgs.transpose(0, 3, 1, 2)

    wl, biases = [], np.zeros((64, 4), np.float32)
    for i in range(4):
        wf, bf = _fold_bn(enc[i])
        ci = wf.shape[2]
        lhsT = wf.transpose(0, 1, 2, 3).reshape(9, ci, 64)
        if i == 0:
            arr = _r11(lhsT.reshape(27, 64))
        else:
            arr = _r11(lhsT.transpose(1, 0, 2).reshape(ci, 9 * 64)
                       .reshape(ci, 9, 64).transpose(0, 1, 2).reshape(ci, 9 * 64))
            # layout [64, tap*64+co]: rows=cin, cols grouped by tap
            arr = _r11(np.concatenate([lhsT[t] for t in range(9)], axis=1))
        wl.append(arr)
        biases[:, i] = bf

    key = "nc"
    if key not in _NC_CACHE:
        _NC_CACHE[key] = _build_bass()
    nc = _NC_CACHE[key]

    in_maps = []
    for c in range(NCORES):
        in_maps.append(_prep_core_inputs(pad[c * PCORE:(c + 1) * PCORE], wl, biases))
    res = run_bass_kernel_spmd(nc, in_maps, list(range(NCORES)))

    emb_dev = np.concatenate([r["emb"].reshape(64, PCORE, 25) for r in res.results], axis=1)
    emb = emb_dev[:, :NIMG].transpose(1, 2, 0).reshape(NIMG, 1600).astype(np.float64)
    return _graph_host(emb, params)
